# revision 13
# baseline (speedup 1.0000x reference)
"""NTM memory update (scatter_memory) on 8 Trainium2 NeuronCores.

Contract: kernel(**inputs) takes the FULL inputs (as produced by the problem's
setup), returns the FULL new_memory [4, 65536, 256] f32.

Strategy:
- The addressing pipeline (cosine over 16 representative rows -> scatter by
  std_ids -> interpolate -> 3-tap circular shift -> sharpen) only touches
  O(N) floats per batch; computed on host in float32 mirroring the reference.
- The memory-regime part, new_memory = memory * (1 - w e^T) + w a^T, moves
  512 MiB through HBM and runs on the 8 cores: shard = (batch, N-half),
  [32768, 256] f32 per core.
- Per super-tile of 1024 rows ([128 partitions, 8 blocks of 128 rows, 256]):
    PE (bf16):  C1 = 1 - w x e  -> PSUM (4 banks, double-buffered)
                (K=9 matmul: ones row + one w row per block; the w x e term
                 is a ~1e-4-scale correction so bf16 operands are safe)
    ACT:        WA_j = a * w_j  (activation Copy with per-partition scale,
                 depends only on constants -> runs arbitrarily ahead)
    DVE:        U   = mem * C1   (tensor_tensor, PSUM operand)
                out = U + WA     (tensor_tensor)
    DMA: input stream on the sync HWDGE queue, output on the scalar queue.
"""

import numpy as np

B, N, M, C = 4, 65536, 256, 16
EPS = 1e-16
NCORES = 8
NSH = N // 2          # rows per core shard
P = 128               # SBUF partitions
JB = 8                # 128-row blocks per super-tile
NT = NSH // P         # 256 blocks per core
ST = NT // JB         # 32 super-tiles per core
K = 1 + JB            # matmul contraction: ones row + one w row per block
F = JB * M            # free size per super-tile (2048)

_CACHE = {}


def _build():
    import concourse.bacc as bacc
    import concourse.tile as tile
    from concourse import mybir

    f32 = mybir.dt.float32
    bf16 = mybir.dt.bfloat16

    nc = bacc.Bacc(
        "TRN2", target_bir_lowering=False, debug=False, num_devices=NCORES
    )
    mem = nc.dram_tensor("mem", [NSH, M], f32, kind="ExternalInput").ap()
    lhs = nc.dram_tensor("lhs", [ST, K, P], bf16, kind="ExternalInput").ap()
    rhs = nc.dram_tensor("rhs", [K, F], bf16, kind="ExternalInput").ap()
    wt = nc.dram_tensor("wt", [P, NT], f32, kind="ExternalInput").ap()
    ab = nc.dram_tensor("ab", [P, M], f32, kind="ExternalInput").ap()
    out = nc.dram_tensor("out", [NSH, M], f32, kind="ExternalOutput").ap()

    memv = mem.rearrange("(s j p) m -> s p j m", j=JB, p=P)
    outv = out.rearrange("(s j p) m -> s p j m", j=JB, p=P)

    NMM = F // 512  # 512-wide matmuls per C1 tile

    with tile.TileContext(nc) as tc:
        with (
            tc.tile_pool(name="const", bufs=1) as cpool,
            tc.tile_pool(name="lhsp", bufs=8) as lpool,
            tc.tile_pool(name="memp", bufs=6) as mpool,
            tc.tile_pool(name="wap", bufs=6) as wapool,
            tc.tile_pool(name="outp", bufs=6) as opool,
            tc.tile_pool(name="ps1", bufs=2, space="PSUM") as ps1pool,
        ):
            R = cpool.tile([K, F], bf16)
            nc.sync.dma_start(R[:], rhs[:])
            W = cpool.tile([P, NT], f32)
            nc.sync.dma_start(W[:], wt[:])
            AB = cpool.tile([P, M], f32)
            nc.sync.dma_start(AB[:], ab[:])

            for s in range(ST):
                mt = mpool.tile([P, JB, M], f32)
                nc.scalar.dma_start(mt[:], memv[s])
                L = lpool.tile([K, P], bf16)
                nc.gpsimd.dma_start(L[:], lhs[s])

                c1 = ps1pool.tile([P, F], f32)
                for q in range(NMM):
                    sl = slice(q * 512, (q + 1) * 512)
                    nc.tensor.matmul(c1[:, sl], L[:], R[:, sl])

                wa = wapool.tile([P, JB, M], f32)
                for j in range(JB):
                    t = s * JB + j
                    nc.scalar.mul(wa[:, j, :], AB[:], W[:, t : t + 1])

                ot = opool.tile([P, JB, M], f32)
                otf = ot.rearrange("p j m -> p (j m)")
                mtf = mt.rearrange("p j m -> p (j m)")
                waf = wa.rearrange("p j m -> p (j m)")
                nc.vector.tensor_mul(otf, mtf, c1[:])
                nc.vector.tensor_add(otf, otf, waf)
                nc.sync.dma_start(outv[s], ot[:])

    nc.compile()
    return nc


def _host_w(memory, k, beta, g, s, gamma, w_prev, std_ids, repre_ids):
    """Addressing pipeline in float32, mirroring the reference op-for-op."""
    f = np.float32
    memory = np.asarray(memory, f)
    k = np.asarray(k, f)
    beta = np.asarray(beta, f)
    g = np.asarray(g, f)
    s = np.asarray(s, f)
    gamma = np.asarray(gamma, f)
    w_prev = np.asarray(w_prev, f)
    std_ids = np.asarray(std_ids)
    repre_ids = np.asarray(repre_ids)

    eps = f(EPS)
    repre = memory[:, repre_ids, :]                          # [B, C, M]
    num = ((repre + eps) * (k[:, None, :] + eps)).sum(-1)    # [B, C]
    den = np.maximum(
        np.linalg.norm(repre + eps, axis=-1)
        * np.linalg.norm(k + eps, axis=-1, keepdims=True),
        f(1e-8),
    ).astype(f)
    cos = beta * np.maximum(num / den, f(0.0)) + eps         # [B, C]
    wc = cos[:, std_ids]                                     # [B, N]
    wg = g * wc + (f(1.0) - g) * w_prev
    wp = np.concatenate([wg[:, -1:], wg, wg[:, :1]], axis=1)
    w_sh = (
        s[:, 0:1] * wp[:, :N]
        + s[:, 1:2] * wp[:, 1 : N + 1]
        + s[:, 2:3] * wp[:, 2 : N + 2]
    )
    wpow = (w_sh.astype(f)) ** gamma
    w = wpow / (wpow.sum(axis=1, keepdims=True) + eps)
    return w.astype(f)                                       # [B, N]


def kernel(memory, k, beta, g, s, gamma, w_prev, e, a, std_ids, repre_ids):
    import ml_dtypes
    from concourse.bass_utils import run_bass_kernel_spmd

    bf16 = ml_dtypes.bfloat16
    memory = np.asarray(memory, np.float32)
    e = np.asarray(e, np.float32)
    a = np.asarray(a, np.float32)

    w = _host_w(memory, k, beta, g, s, gamma, w_prev, std_ids, repre_ids)

    if "nc" not in _CACHE:
        _CACHE["nc"] = _build()
    nc = _CACHE["nc"]

    in_maps = []
    for c in range(NCORES):
        b, h = divmod(c, 2)
        mem_shard = np.ascontiguousarray(memory[b, h * NSH : (h + 1) * NSH, :])
        w_shard = w[b, h * NSH : (h + 1) * NSH]

        lhs = np.zeros((ST, K, P), np.float32)
        lhs[:, 0, :] = 1.0
        lhs[:, 1:, :] = w_shard.reshape(ST, JB, P)  # [s, j, p]

        rhs = np.zeros((K, F), np.float32)
        rhs[0, :] = 1.0
        for j in range(JB):
            rhs[1 + j, j * M : (j + 1) * M] = -e[b]

        in_maps.append(
            {
                "mem": mem_shard,
                "lhs": lhs.astype(bf16),
                "rhs": rhs.astype(bf16),
                "wt": np.ascontiguousarray(w_shard.reshape(NT, P).T),
                "ab": np.ascontiguousarray(np.broadcast_to(a[b], (P, M))),
            }
        )

    try:
        res = run_bass_kernel_spmd(nc, in_maps, core_ids=list(range(NCORES)))
    except Exception:
        # one retry for transient device/runtime hiccups
        res = run_bass_kernel_spmd(nc, in_maps, core_ids=list(range(NCORES)))
    _CACHE["last_result"] = res

    new_memory = np.empty((B, N, M), np.float32)
    for c in range(NCORES):
        b, h = divmod(c, 2)
        new_memory[b, h * NSH : (h + 1) * NSH, :] = res.results[c]["out"]
    return new_memory


# revision 14
# speedup vs baseline: 1.1845x; 1.1845x over previous
"""NTM memory update (scatter_memory) on 8 Trainium2 NeuronCores.

Contract: kernel(**inputs) takes the FULL inputs (as produced by the problem's
setup), returns the FULL new_memory [4, 65536, 256] f32.

Strategy:
- The addressing pipeline (cosine over 16 representative rows -> scatter by
  std_ids -> interpolate -> 3-tap circular shift -> sharpen) only touches
  O(N) floats per batch; computed on host in float32 mirroring the reference.
- The memory-regime part, new_memory = memory * (1 - w e^T) + w a^T, moves
  512 MiB through HBM and runs on the 8 cores: shard = (batch, N-half),
  [32768, 256] f32 per core.
- Per super-tile of 1024 rows ([128 partitions, 8 blocks of 128 rows, 256]):
    PE (bf16):  C1 = 1 - w x e  -> PSUM (4 banks, double-buffered)
                (K=9 matmul: ones row + one w row per block; the w x e term
                 is a ~1e-4-scale correction so bf16 operands are safe)
    ACT:        WA_j = a * w_j  (activation Copy with per-partition scale,
                 depends only on constants -> runs arbitrarily ahead)
    DVE:        U   = mem * C1   (tensor_tensor, PSUM operand)
                out = U + WA     (tensor_tensor)
    DMA: input stream on the sync HWDGE queue, output on the scalar queue.
"""

import numpy as np

B, N, M, C = 4, 65536, 256, 16
EPS = 1e-16
NCORES = 8
NSH = N // 2          # rows per core shard
P = 128               # SBUF partitions
JB = 8                # 128-row blocks per super-tile
NT = NSH // P         # 256 blocks per core
ST = NT // JB         # 32 super-tiles per core
K = 1 + JB            # matmul contraction: ones row + one w row per block
F = JB * M            # free size per super-tile (2048)

_CACHE = {}


def _build():
    import concourse.bacc as bacc
    import concourse.tile as tile
    from concourse import mybir

    f32 = mybir.dt.float32
    bf16 = mybir.dt.bfloat16

    nc = bacc.Bacc(
        "TRN2", target_bir_lowering=False, debug=False, num_devices=NCORES
    )
    mem = nc.dram_tensor("mem", [NSH, M], f32, kind="ExternalInput").ap()
    lhs = nc.dram_tensor("lhs", [ST, K, P], bf16, kind="ExternalInput").ap()
    rhs = nc.dram_tensor("rhs", [K, F], bf16, kind="ExternalInput").ap()
    wt = nc.dram_tensor("wt", [P, NT], f32, kind="ExternalInput").ap()
    ab = nc.dram_tensor("ab", [P, M], f32, kind="ExternalInput").ap()
    out = nc.dram_tensor("out", [NSH, M], f32, kind="ExternalOutput").ap()

    memv = mem.rearrange("(s j p) m -> s p j m", j=JB, p=P)
    outv = out.rearrange("(s j p) m -> s p j m", j=JB, p=P)

    NMM = F // 512  # 512-wide matmuls per C1 tile

    with tile.TileContext(nc) as tc:
        with (
            tc.tile_pool(name="const", bufs=1) as cpool,
            tc.tile_pool(name="lhsp", bufs=8) as lpool,
            tc.tile_pool(name="memp", bufs=8) as mpool,
            tc.tile_pool(name="wap", bufs=4) as wapool,
            tc.tile_pool(name="outp", bufs=6) as opool,
            tc.tile_pool(name="ps1", bufs=2, space="PSUM") as ps1pool,
        ):
            R = cpool.tile([K, F], bf16)
            nc.sync.dma_start(R[:], rhs[:])
            W = cpool.tile([P, NT], f32)
            nc.sync.dma_start(W[:], wt[:])
            AB = cpool.tile([P, M], f32)
            nc.sync.dma_start(AB[:], ab[:])

            for s in range(ST):
                mt = mpool.tile([P, JB, M], f32)
                nc.scalar.dma_start(mt[:], memv[s])
                L = lpool.tile([K, P], bf16)
                nc.gpsimd.dma_start(L[:], lhs[s])

                c1 = ps1pool.tile([P, F], f32)
                for q in range(NMM):
                    sl = slice(q * 512, (q + 1) * 512)
                    nc.tensor.matmul(c1[:, sl], L[:], R[:, sl])

                wa = wapool.tile([P, JB, M], f32)
                for j in range(JB):
                    t = s * JB + j
                    nc.scalar.mul(wa[:, j, :], AB[:], W[:, t : t + 1])

                ot = opool.tile([P, JB, M], f32)
                otf = ot.rearrange("p j m -> p (j m)")
                mtf = mt.rearrange("p j m -> p (j m)")
                waf = wa.rearrange("p j m -> p (j m)")
                nc.vector.tensor_mul(otf, mtf, c1[:])
                nc.vector.tensor_add(otf, otf, waf)
                nc.sync.dma_start(outv[s], ot[:])

    nc.compile()
    return nc


def _host_w(memory, k, beta, g, s, gamma, w_prev, std_ids, repre_ids):
    """Addressing pipeline in float32, mirroring the reference op-for-op."""
    f = np.float32
    memory = np.asarray(memory, f)
    k = np.asarray(k, f)
    beta = np.asarray(beta, f)
    g = np.asarray(g, f)
    s = np.asarray(s, f)
    gamma = np.asarray(gamma, f)
    w_prev = np.asarray(w_prev, f)
    std_ids = np.asarray(std_ids)
    repre_ids = np.asarray(repre_ids)

    eps = f(EPS)
    repre = memory[:, repre_ids, :]                          # [B, C, M]
    num = ((repre + eps) * (k[:, None, :] + eps)).sum(-1)    # [B, C]
    den = np.maximum(
        np.linalg.norm(repre + eps, axis=-1)
        * np.linalg.norm(k + eps, axis=-1, keepdims=True),
        f(1e-8),
    ).astype(f)
    cos = beta * np.maximum(num / den, f(0.0)) + eps         # [B, C]
    wc = cos[:, std_ids]                                     # [B, N]
    wg = g * wc + (f(1.0) - g) * w_prev
    wp = np.concatenate([wg[:, -1:], wg, wg[:, :1]], axis=1)
    w_sh = (
        s[:, 0:1] * wp[:, :N]
        + s[:, 1:2] * wp[:, 1 : N + 1]
        + s[:, 2:3] * wp[:, 2 : N + 2]
    )
    wpow = (w_sh.astype(f)) ** gamma
    w = wpow / (wpow.sum(axis=1, keepdims=True) + eps)
    return w.astype(f)                                       # [B, N]


def kernel(memory, k, beta, g, s, gamma, w_prev, e, a, std_ids, repre_ids):
    import ml_dtypes
    from concourse.bass_utils import run_bass_kernel_spmd

    bf16 = ml_dtypes.bfloat16
    memory = np.asarray(memory, np.float32)
    e = np.asarray(e, np.float32)
    a = np.asarray(a, np.float32)

    w = _host_w(memory, k, beta, g, s, gamma, w_prev, std_ids, repre_ids)

    if "nc" not in _CACHE:
        _CACHE["nc"] = _build()
    nc = _CACHE["nc"]

    in_maps = []
    for c in range(NCORES):
        b, h = divmod(c, 2)
        mem_shard = np.ascontiguousarray(memory[b, h * NSH : (h + 1) * NSH, :])
        w_shard = w[b, h * NSH : (h + 1) * NSH]

        lhs = np.zeros((ST, K, P), np.float32)
        lhs[:, 0, :] = 1.0
        lhs[:, 1:, :] = w_shard.reshape(ST, JB, P)  # [s, j, p]

        rhs = np.zeros((K, F), np.float32)
        rhs[0, :] = 1.0
        for j in range(JB):
            rhs[1 + j, j * M : (j + 1) * M] = -e[b]

        in_maps.append(
            {
                "mem": mem_shard,
                "lhs": lhs.astype(bf16),
                "rhs": rhs.astype(bf16),
                "wt": np.ascontiguousarray(w_shard.reshape(NT, P).T),
                "ab": np.ascontiguousarray(np.broadcast_to(a[b], (P, M))),
            }
        )

    try:
        res = run_bass_kernel_spmd(nc, in_maps, core_ids=list(range(NCORES)))
    except Exception:
        # one retry for transient device/runtime hiccups
        res = run_bass_kernel_spmd(nc, in_maps, core_ids=list(range(NCORES)))
    _CACHE["last_result"] = res

    new_memory = np.empty((B, N, M), np.float32)
    for c in range(NCORES):
        b, h = divmod(c, 2)
        new_memory[b, h * NSH : (h + 1) * NSH, :] = res.results[c]["out"]
    return new_memory


# revision 15
# speedup vs baseline: 1.1861x; 1.0013x over previous
"""NTM memory update (scatter_memory) on 8 Trainium2 NeuronCores.

Contract: kernel(**inputs) takes the FULL inputs (as produced by the problem's
setup), returns the FULL new_memory [4, 65536, 256] f32.

Strategy:
- The addressing pipeline (cosine over 16 representative rows -> scatter by
  std_ids -> interpolate -> 3-tap circular shift -> sharpen) only touches
  O(N) floats per batch; computed on host in float32 mirroring the reference.
- The memory-regime part, new_memory = memory * (1 - w e^T) + w a^T, moves
  512 MiB through HBM and runs on the 8 cores: shard = (batch, N-half),
  [32768, 256] f32 per core.
- Per super-tile of 1024 rows ([128 partitions, 8 blocks of 128 rows, 256]):
    PE (bf16):  C1 = 1 - w x e  -> PSUM (4 banks, double-buffered)
                (K=9 matmul: ones row + one w row per block; the w x e term
                 is a ~1e-4-scale correction so bf16 operands are safe)
    ACT:        WA_j = a * w_j  (activation Copy with per-partition scale,
                 depends only on constants -> runs arbitrarily ahead)
    DVE:        U   = mem * C1   (tensor_tensor, PSUM operand)
                out = U + WA     (tensor_tensor)
  DMA queues: input stream on the scalar HWDGE queue (no upstream deps, so
  its triggers never stall the FIFO), output on the sync queue, small lhs
  tiles on gpsimd. Input tiles are prefetched 8 deep to ride out HBM-stack
  contention with the paired core.
"""

import numpy as np

B, N, M, C = 4, 65536, 256, 16
EPS = 1e-16
NCORES = 8
NSH = N // 2          # rows per core shard
P = 128               # SBUF partitions
JB = 8                # 128-row blocks per super-tile
NT = NSH // P         # 256 blocks per core
ST = NT // JB         # 32 super-tiles per core
K = 1 + JB            # matmul contraction: ones row + one w row per block
F = JB * M            # free size per super-tile (2048)

_CACHE = {}


def _build():
    import concourse.bacc as bacc
    import concourse.tile as tile
    from concourse import mybir

    f32 = mybir.dt.float32
    bf16 = mybir.dt.bfloat16

    nc = bacc.Bacc(
        "TRN2", target_bir_lowering=False, debug=False, num_devices=NCORES
    )
    mem = nc.dram_tensor("mem", [NSH, M], f32, kind="ExternalInput").ap()
    lhs = nc.dram_tensor("lhs", [ST, K, P], bf16, kind="ExternalInput").ap()
    rhs = nc.dram_tensor("rhs", [K, F], bf16, kind="ExternalInput").ap()
    wt = nc.dram_tensor("wt", [P, NT], f32, kind="ExternalInput").ap()
    ab = nc.dram_tensor("ab", [P, M], f32, kind="ExternalInput").ap()
    out = nc.dram_tensor("out", [NSH, M], f32, kind="ExternalOutput").ap()

    memv = mem.rearrange("(s j p) m -> s p j m", j=JB, p=P)
    outv = out.rearrange("(s j p) m -> s p j m", j=JB, p=P)

    NMM = F // 512  # 512-wide matmuls per C1 tile

    with tile.TileContext(nc) as tc:
        with (
            tc.tile_pool(name="const", bufs=1) as cpool,
            tc.tile_pool(name="lhsp", bufs=8) as lpool,
            tc.tile_pool(name="memp", bufs=8) as mpool,
            tc.tile_pool(name="wap", bufs=4) as wapool,
            tc.tile_pool(name="outp", bufs=6) as opool,
            tc.tile_pool(name="ps1", bufs=2, space="PSUM") as ps1pool,
        ):
            R = cpool.tile([K, F], bf16)
            nc.sync.dma_start(R[:], rhs[:])
            W = cpool.tile([P, NT], f32)
            nc.sync.dma_start(W[:], wt[:])
            AB = cpool.tile([P, M], f32)
            nc.sync.dma_start(AB[:], ab[:])

            for s in range(ST):
                mt = mpool.tile([P, JB, M], f32)
                nc.scalar.dma_start(mt[:], memv[s])
                L = lpool.tile([K, P], bf16)
                nc.gpsimd.dma_start(L[:], lhs[s])

                c1 = ps1pool.tile([P, F], f32)
                for q in range(NMM):
                    sl = slice(q * 512, (q + 1) * 512)
                    nc.tensor.matmul(c1[:, sl], L[:], R[:, sl])

                wa = wapool.tile([P, JB, M], f32)
                for j in range(JB):
                    t = s * JB + j
                    nc.scalar.mul(wa[:, j, :], AB[:], W[:, t : t + 1])

                ot = opool.tile([P, JB, M], f32)
                otf = ot.rearrange("p j m -> p (j m)")
                mtf = mt.rearrange("p j m -> p (j m)")
                waf = wa.rearrange("p j m -> p (j m)")
                nc.vector.tensor_mul(otf, mtf, c1[:])
                nc.vector.tensor_add(otf, otf, waf)
                nc.sync.dma_start(outv[s], ot[:])

    nc.compile()
    return nc


def _host_w(memory, k, beta, g, s, gamma, w_prev, std_ids, repre_ids):
    """Addressing pipeline in float32, mirroring the reference op-for-op."""
    f = np.float32
    memory = np.asarray(memory, f)
    k = np.asarray(k, f)
    beta = np.asarray(beta, f)
    g = np.asarray(g, f)
    s = np.asarray(s, f)
    gamma = np.asarray(gamma, f)
    w_prev = np.asarray(w_prev, f)
    std_ids = np.asarray(std_ids)
    repre_ids = np.asarray(repre_ids)

    eps = f(EPS)
    repre = memory[:, repre_ids, :]                          # [B, C, M]
    num = ((repre + eps) * (k[:, None, :] + eps)).sum(-1)    # [B, C]
    den = np.maximum(
        np.linalg.norm(repre + eps, axis=-1)
        * np.linalg.norm(k + eps, axis=-1, keepdims=True),
        f(1e-8),
    ).astype(f)
    cos = beta * np.maximum(num / den, f(0.0)) + eps         # [B, C]
    wc = cos[:, std_ids]                                     # [B, N]
    wg = g * wc + (f(1.0) - g) * w_prev
    wp = np.concatenate([wg[:, -1:], wg, wg[:, :1]], axis=1)
    w_sh = (
        s[:, 0:1] * wp[:, :N]
        + s[:, 1:2] * wp[:, 1 : N + 1]
        + s[:, 2:3] * wp[:, 2 : N + 2]
    )
    wpow = (w_sh.astype(f)) ** gamma
    w = wpow / (wpow.sum(axis=1, keepdims=True) + eps)
    return w.astype(f)                                       # [B, N]


def kernel(memory, k, beta, g, s, gamma, w_prev, e, a, std_ids, repre_ids):
    import ml_dtypes
    from concourse.bass_utils import run_bass_kernel_spmd

    bf16 = ml_dtypes.bfloat16
    memory = np.asarray(memory, np.float32)
    e = np.asarray(e, np.float32)
    a = np.asarray(a, np.float32)

    w = _host_w(memory, k, beta, g, s, gamma, w_prev, std_ids, repre_ids)

    if "nc" not in _CACHE:
        _CACHE["nc"] = _build()
    nc = _CACHE["nc"]

    in_maps = []
    for c in range(NCORES):
        b, h = divmod(c, 2)
        mem_shard = np.ascontiguousarray(memory[b, h * NSH : (h + 1) * NSH, :])
        w_shard = w[b, h * NSH : (h + 1) * NSH]

        lhs = np.zeros((ST, K, P), np.float32)
        lhs[:, 0, :] = 1.0
        lhs[:, 1:, :] = w_shard.reshape(ST, JB, P)  # [s, j, p]

        rhs = np.zeros((K, F), np.float32)
        rhs[0, :] = 1.0
        for j in range(JB):
            rhs[1 + j, j * M : (j + 1) * M] = -e[b]

        in_maps.append(
            {
                "mem": mem_shard,
                "lhs": lhs.astype(bf16),
                "rhs": rhs.astype(bf16),
                "wt": np.ascontiguousarray(w_shard.reshape(NT, P).T),
                "ab": np.ascontiguousarray(np.broadcast_to(a[b], (P, M))),
            }
        )

    try:
        res = run_bass_kernel_spmd(nc, in_maps, core_ids=list(range(NCORES)))
    except Exception:
        # one retry for transient device/runtime hiccups
        res = run_bass_kernel_spmd(nc, in_maps, core_ids=list(range(NCORES)))
    _CACHE["last_result"] = res

    new_memory = np.empty((B, N, M), np.float32)
    for c in range(NCORES):
        b, h = divmod(c, 2)
        new_memory[b, h * NSH : (h + 1) * NSH, :] = res.results[c]["out"]
    return new_memory
